# revision 1
# baseline (speedup 1.0000x reference)
"""Trainium2 Bass kernel for a dense transformer block.

Problem: B=4, N=1024, C=1024, H=16 heads (HD=64), MLP hidden 4096, pre-norm,
RoPE on q/k, exact gelu.

Sharding (8 cores, no collectives): core c handles batch b=c//2 and
sequence-half h=c%2. Each core computes LN1 + K/V over its batch's full 1024
tokens (cheap duplication), and Q / attention / proj / MLP only for its 512
local tokens. Tokens are permuted per-core so the local half is always
columns 0:512 -> all cores run an identical program.

On-chip layout is feature-major (transposed): activations live as [C_part,
token_free] so weights are used directly as stationary matmul operands
(lhsT) and activations stream as the moving operand (N=512, float32r ->
full PE rate). The host pre-transposes x, pre-tiles all weights into
[out_tile][128, kchunks*128] blocks, and pre-permutes w_q/w_k columns into a
[re(h)|im(h)|re(h')|im(h')] head-pair layout.

RoPE: out = in*cosR + blockswap(in*sinPM), where sinPM carries the +/- sign
per 32-row block and blockswap is 4 cross-partition GpSimd copies.

Attention per head-pair tile j (heads 2j, 2j+1): scoresT[k,q] =
(k^T chunk).T @ q^T via single K=64 matmuls (head dims contiguous on
partitions 0:64 / 64:128); exp on ScalarE straight out of PSUM (scale=1/8
folded in); MM2 with lhsT=[v | ones32] (M=96, K=128 accumulation over
k-chunks) yields o_unnorm on partitions 0:64 and the softmax denominator
replicated 32x on 64:96; normalize via cross-partition ACT copies +
reciprocal + aligned multiply.

LayerNorm (feature-major): column sums via all-ones [128,128] stationary
matmuls accumulated over chunks -> sums already replicated across all 128
partitions; var = E[x^2]-mean^2; apply fused with gamma/beta per-partition.

NOTE: empirically found toolchain constraints this kernel respects:
- every buffer consumed by an fp32r matmul must be produced as float32r
- walrus allows only 1 semaphore wait per instruction (excess waits are
  split onto EventSemaphore carriers by a BIR post-pass below)
- accumulating matmuls (start=False) require K=128 (K<128 accumulation
  faults the device); single matmuls may use any K
- vector.reciprocal must not read PSUM
- tensor_tensor operands must share the start partition; single-input ops
  (copy/activation/reciprocal) may cross partitions
- Memset cannot write float32r tiles (ones come from DRAM instead)
"""

import json
import ml_dtypes
import numpy as np
from contextlib import ExitStack

import concourse.bass as bass
import concourse.tile as tile
from concourse import mybir
from concourse.bass_utils import run_bass_kernel_spmd

_MAXW = 1


def _split_multiwait(bir_bytes):
    """Move excess per-instruction semaphore waits onto same-engine
    EventSemaphore carriers inserted before the instruction (engine queues
    are in-order, so this is semantically identical)."""
    bir = json.loads(bir_bytes)
    n = [0]
    for fn in bir.get("functions", []):
        for bb in fn.get("blocks", []):
            out = []
            for inst in bb.get("instructions", []):
                si = inst.get("sync_info")
                ow = (si or {}).get("on_wait") or []
                if len(ow) > _MAXW:
                    excess, keep = ow[:-_MAXW], ow[-_MAXW:]
                    for s in range(0, len(excess), _MAXW):
                        n[0] += 1
                        out.append({
                            "debug": inst.get("debug", 0),
                            "engine": inst["engine"],
                            "ins": [],
                            "name": f"antsplitw-{n[0]}",
                            "opcode": "EventSemaphore",
                            "outs": [],
                            "sync_info": {"on_update": [],
                                          "on_wait": excess[s:s + _MAXW]},
                        })
                    si["on_wait"] = keep
                out.append(inst)
            bb["instructions"] = out
    return json.dumps(bir).encode()


def _install_multiwait_hook():
    import concourse.bass2jax as bass2jax
    from concourse import bass_utils as bu
    if getattr(bass2jax, "_ant_multiwait_hooked", False):
        return
    orig = bu.compile_bir_kernel

    def wrapper(bir_json, tmpdir, neff_name="file.neff"):
        if isinstance(bir_json, str):
            bir_json = bir_json.encode()
        return orig(_split_multiwait(bir_json), tmpdir, neff_name)

    bass2jax.compile_bir_kernel = wrapper
    bass2jax._ant_multiwait_hooked = True


# ---- problem constants (hardcoded per harness contract) ----
B, N, C, H = 4, 1024, 1024, 16
HD = C // H            # 64
HID = 4 * C            # 4096
EPS = 1e-5
P = 128
KC = C // P            # 8 contraction chunks over C
HJ = HID // P          # 32 chunks over hidden
TQ = N // 2            # 512 local query tokens per core
VW = HD + 32           # v tile width: 64 v dims + 32 ones
NCORES = 8

F32 = mybir.dt.float32
F32R = mybir.dt.float32r
BF16 = mybir.dt.bfloat16
FT = mybir.ActivationFunctionType
OP = mybir.AluOpType


# ----------------------------------------------------------------------------
# Bass program (identical for every core)
# ----------------------------------------------------------------------------

def build_nc(reps=1):
    nc = bass.Bass("TRN2", target_bir_lowering=False, debug=False)

    # -------- DRAM I/O --------
    d_xT = nc.dram_tensor("xT", [C, N], F32R, kind="ExternalInput").ap()
    d_cos = nc.dram_tensor("cosR", [P, N], F32, kind="ExternalInput").ap()
    d_spm = nc.dram_tensor("sinPM", [P, N], F32, kind="ExternalInput").ap()
    d_ones = nc.dram_tensor("onesT", [P, H * 32], F32R, kind="ExternalInput").ap()
    d_wq = nc.dram_tensor("wq", [KC, P, C], BF16, kind="ExternalInput").ap()
    d_wk = nc.dram_tensor("wk", [KC, P, C], BF16, kind="ExternalInput").ap()
    d_wv = nc.dram_tensor("wv", [P, KC, C], BF16, kind="ExternalInput").ap()
    d_wp = nc.dram_tensor("wp", [KC, P, C], BF16, kind="ExternalInput").ap()
    d_wf1 = nc.dram_tensor("wf1", [HJ, P, C], BF16, kind="ExternalInput").ap()
    d_wf2 = nc.dram_tensor("wf2", [KC, P, HID], BF16, kind="ExternalInput").ap()
    d_ln1g = nc.dram_tensor("ln1g", [P, KC], F32, kind="ExternalInput").ap()
    d_ln1b = nc.dram_tensor("ln1b", [P, KC], F32, kind="ExternalInput").ap()
    d_ln2g = nc.dram_tensor("ln2g", [P, KC], F32, kind="ExternalInput").ap()
    d_ln2b = nc.dram_tensor("ln2b", [P, KC], F32, kind="ExternalInput").ap()
    d_bp = nc.dram_tensor("bp", [P, KC], F32, kind="ExternalInput").ap()
    d_bf1 = nc.dram_tensor("bf1", [P, HJ], F32, kind="ExternalInput").ap()
    d_bf2 = nc.dram_tensor("bf2", [P, KC], F32, kind="ExternalInput").ap()
    d_out = nc.dram_tensor("outT", [KC, P, TQ], F32, kind="ExternalOutput").ap()

    xT_t = d_xT.rearrange("(kc p) t -> p kc t", p=P)  # [128, 8, 1024]

    with tile.TileContext(nc) as tc, ExitStack() as top:
        const = top.enter_context(tc.tile_pool(name="const", bufs=1))

        # ---- constants ----
        eps_t = const.tile([P, 1], F32, tag="eps")
        nc.vector.memset(eps_t, EPS)
        ones128 = const.tile([P, P], F32R, tag="ones128")
        nc.sync.dma_start(out=ones128, in_=d_ones[:, 0:P])

        def load_const(name, dram, cols):
            t = const.tile([P, cols], F32, tag=name)
            nc.sync.dma_start(out=t, in_=dram)
            return t

        ln1g = load_const("ln1g", d_ln1g, KC)
        ln1b = load_const("ln1b", d_ln1b, KC)
        ln2g = load_const("ln2g", d_ln2g, KC)
        ln2b = load_const("ln2b", d_ln2b, KC)
        bp = load_const("bp", d_bp, KC)
        bf1 = load_const("bf1", d_bf1, HJ)
        bf2 = load_const("bf2", d_bf2, KC)

        def emit(rep):
            big = tc.alloc_tile_pool(name=f"big{rep}", bufs=1)
            # ---- long-lived activations ----
            # t16a slot: xloc (phases A-D), then h2 (E-F)
            xloc = big.tile([P, KC, TQ], F32R, tag="t16a")
            nc.sync.dma_start(out=xloc, in_=xT_t[:, :, 0:TQ])
            osb = big.tile([P, KC, TQ], BF16, tag="osb")       # attention out (o^T)
            resid = big.tile([P, KC, TQ], F32R, tag="resid")   # x + attn

            # feature-major layernorm: mean/rstd replicated on all 128 partitions
            def ln_stats(src_tiles, width, psumpool, wk, m_rep, r_rep):
                """src_tiles(kc, half) -> [128, 512] AP over `width` tokens.
                Fills m_rep/r_rep [128, width] (rows identical)."""
                for hf in range(width // 512):
                    sl = slice(hf * 512, hf * 512 + 512)
                    ps_s = psumpool.tile([P, 512], F32, tag="ps_stat_s")
                    ps_q = psumpool.tile([P, 512], F32, tag="ps_stat_q")
                    for kc in range(KC):
                        xpart = src_tiles(kc, hf)
                        nc.tensor.matmul(ps_s, lhsT=ones128, rhs=xpart,
                                         start=(kc == 0), stop=(kc == KC - 1))
                        sq = wk.tile([P, 512], F32R, tag="ln_sq")
                        nc.vector.tensor_mul(sq, xpart, xpart)
                        nc.tensor.matmul(ps_q, lhsT=ones128, rhs=sq,
                                         start=(kc == 0), stop=(kc == KC - 1))
                    nc.scalar.mul(m_rep[:, sl], ps_s, 1.0 / C)
                    qrep = wk.tile([P, 512], F32, tag="ln_qrep")
                    nc.scalar.mul(qrep, ps_q, 1.0 / C)
                    # var = E[x^2] - mean^2; rstd = 1/sqrt(var + eps)
                    vrep = wk.tile([P, 512], F32, tag="ln_vrep")
                    nc.vector.tensor_mul(vrep, m_rep[:, sl], m_rep[:, sl])
                    nc.vector.tensor_sub(vrep, qrep, vrep)
                    nc.scalar.activation(vrep, vrep, FT.Sqrt, bias=eps_t)
                    nc.vector.reciprocal(r_rep[:, sl], vrep)

            # phase-A pool on the right side (non-LIFO release vs attention pool)
            phA_cm = tc.tile_pool(name=f"phA{rep}", bufs=1, side="right")
            pA = phA_cm.__enter__()
            h1 = pA.tile([P, KC, N], BF16, tag="h1")           # LN1 out (32KB/part)
            cosR = pA.tile([P, N], F32, tag="cosR")
            nc.sync.dma_start(out=cosR, in_=d_cos)
            sinPM = pA.tile([P, N], F32, tag="sinPM")
            nc.sync.dma_start(out=sinPM, in_=d_spm)

            # ================= Phase A: LN1 over all 1024 tokens =================
            with ExitStack() as phA:
                wkA = phA.enter_context(tc.tile_pool(name=f"wkA{rep}", bufs=3))
                psA = phA.enter_context(tc.tile_pool(name=f"psA{rep}", bufs=1, space="PSUM"))
                xrp = phA.enter_context(tc.tile_pool(name=f"xrp{rep}", bufs=1))
                xrem = xrp.tile([P, KC, TQ], F32R, tag="xrem")
                nc.sync.dma_start(out=xrem, in_=xT_t[:, :, TQ:N])
                m1 = xrp.tile([P, N], F32, tag="m1rep")
                r1 = xrp.tile([P, N], F32, tag="r1rep")

                def src1(kc, hf):
                    return xloc[:, kc, :] if hf == 0 else xrem[:, kc, :]

                ln_stats(src1, N, psA, wkA, m1, r1)
                # apply: h1 = (x - m) * r * g + b
                for kc in range(KC):
                    for hf in range(2):
                        sl = slice(hf * 512, hf * 512 + 512)
                        t1 = wkA.tile([P, 512], F32, tag="ln_t1")
                        nc.vector.tensor_sub(t1, src1(kc, hf), m1[:, sl])
                        nc.vector.tensor_mul(t1, t1, r1[:, sl])
                        nc.vector.tensor_scalar(
                            out=h1[:, kc, sl], in0=t1,
                            scalar1=ln1g[:, kc:kc + 1], scalar2=ln1b[:, kc:kc + 1],
                            op0=OP.mult, op1=OP.add)

            # attention-span pool (opens before phA closes; closed after attention)
            attn_cm = tc.tile_pool(name=f"attn{rep}", bufs=1)
            pAT = attn_cm.__enter__()
            # vsb[p, tj, head, 0:64] = v[token tj*128+p, head*64+d]
            # vsb[p, tj, head, 64:96] = 1.0  (softmax-denominator trick)
            vsb = pAT.tile([P, KC, H, VW], F32R, tag="vsb")    # 48KB/part
            qsb = pAT.tile([P, KC, TQ], F32R, tag="qsb")
            ksb = pAT.tile([P, KC, N], F32R, tag="ksb")
            for tj in range(KC):
                nc.sync.dma_start(
                    out=vsb[:, tj, :, HD:VW],
                    in_=d_ones.rearrange("p (h w) -> p h w", h=H))

            # ================= Phase B1: V = h1 @ wv (token-major) ===============
            with ExitStack() as phB1:
                wvp = phB1.enter_context(tc.tile_pool(name=f"wvp{rep}", bufs=2))
                psB1 = phB1.enter_context(tc.tile_pool(name=f"psB1{rep}", bufs=3, space="PSUM"))
                for hf in range(4):
                    wvt = wvp.tile([P, KC, 256], BF16, tag="wvt")
                    nc.sync.dma_start(out=wvt, in_=d_wv[:, :, hf * 256:hf * 256 + 256])
                    for tj in range(KC):
                        ps_v = psB1.tile([P, 256], F32, tag="ps_v")
                        for kc in range(KC):
                            nc.tensor.matmul(
                                ps_v,
                                lhsT=h1[:, kc, tj * P:(tj + 1) * P],
                                rhs=wvt[:, kc, :],
                                start=(kc == 0), stop=(kc == KC - 1))
                        nc.scalar.copy(
                            vsb[:, tj, hf * 4:(hf + 1) * 4, 0:HD],
                            ps_v.rearrange("p (h d) -> p h d", h=4))

            # ================= Phase B2: Q/K + RoPE ==============================
            def rope(out_ap, ps, cosA, spmA, width, wk):
                tcos = wk.tile([P, width], F32, tag="ropec")
                tpm = wk.tile([P, width], F32, tag="ropes")
                nc.vector.tensor_mul(tcos, ps, cosA)
                nc.vector.tensor_mul(tpm, ps, spmA)
                tsh = wk.tile([P, width], F32, tag="ropesh")
                nc.gpsimd.tensor_copy(tsh[0:32, :], tpm[32:64, :])
                nc.gpsimd.tensor_copy(tsh[32:64, :], tpm[0:32, :])
                nc.gpsimd.tensor_copy(tsh[64:96, :], tpm[96:128, :])
                nc.gpsimd.tensor_copy(tsh[96:128, :], tpm[64:96, :])
                nc.vector.tensor_add(out_ap, tcos, tsh)

            with ExitStack() as phB2:
                wqp = phB2.enter_context(tc.tile_pool(name=f"wqp{rep}", bufs=2))
                wkB = phB2.enter_context(tc.tile_pool(name=f"wkB{rep}", bufs=2))
                psB2 = phB2.enter_context(tc.tile_pool(name=f"psB2{rep}", bufs=3, space="PSUM"))
                for fj in range(KC):
                    wt = wqp.tile([P, KC, P], BF16, tag="wqkv")
                    nc.sync.dma_start(
                        out=wt, in_=d_wq[fj].rearrange("p (kc f) -> p kc f", kc=KC))
                    ps_q = psB2.tile([P, 512], F32, tag="ps_qk")
                    for kc in range(KC):
                        nc.tensor.matmul(ps_q, lhsT=wt[:, kc, :],
                                         rhs=h1[:, kc, 0:TQ],
                                         start=(kc == 0), stop=(kc == KC - 1))
                    rope(qsb[:, fj, :], ps_q, cosR[:, 0:TQ], sinPM[:, 0:TQ], TQ, wkB)
                for fj in range(KC):
                    wt = wqp.tile([P, KC, P], BF16, tag="wqkv")
                    nc.sync.dma_start(
                        out=wt, in_=d_wk[fj].rearrange("p (kc f) -> p kc f", kc=KC))
                    for hf in range(2):
                        sl = slice(hf * 512, hf * 512 + 512)
                        ps_k = psB2.tile([P, 512], F32, tag="ps_qk")
                        for kc in range(KC):
                            nc.tensor.matmul(ps_k, lhsT=wt[:, kc, :],
                                             rhs=h1[:, kc, sl],
                                             start=(kc == 0), stop=(kc == KC - 1))
                        rope(ksb[:, fj, sl], ps_k, cosR[:, sl], sinPM[:, sl], 512, wkB)

            phA_cm.__exit__(None, None, None)  # free h1 + trig (40KB/part)

            # ================= Phase C: attention ================================
            with ExitStack() as phC:
                wkC = phC.enter_context(tc.tile_pool(name=f"wkC{rep}", bufs=3))
                psS = phC.enter_context(tc.tile_pool(name=f"psS{rep}", bufs=2, space="PSUM"))
                ps2 = phC.enter_context(tc.tile_pool(name=f"ps2{rep}", bufs=2, space="PSUM"))
                scale = float(HD) ** -0.5
                for j in range(KC):  # head pair j -> heads 2j, 2j+1
                    p2a = ps2.tile([P, TQ], F32, tag="ps2a")
                    p2b = ps2.tile([P, TQ], F32, tag="ps2b")
                    for kc in range(KC):
                        ksl = slice(kc * P, (kc + 1) * P)
                        psa = psS.tile([P, TQ], F32, tag="ps_sa")
                        nc.tensor.matmul(psa, lhsT=ksb[0:HD, j, ksl],
                                         rhs=qsb[0:HD, j, :], start=True, stop=True)
                        psb = psS.tile([P, TQ], F32, tag="ps_sb")
                        nc.tensor.matmul(psb, lhsT=ksb[HD:P, j, ksl],
                                         rhs=qsb[HD:P, j, :], start=True, stop=True)
                        ea = wkC.tile([P, TQ], F32R, tag="expa")
                        nc.scalar.activation(ea, psa, FT.Exp, scale=scale)
                        eb = wkC.tile([P, TQ], F32R, tag="expb")
                        nc.scalar.activation(eb, psb, FT.Exp, scale=scale)
                        nc.tensor.matmul(p2a[0:VW, :], lhsT=vsb[:, kc, 2 * j, :],
                                         rhs=ea, start=(kc == 0), stop=(kc == KC - 1))
                        nc.tensor.matmul(p2b[0:VW, :], lhsT=vsb[:, kc, 2 * j + 1, :],
                                         rhs=eb, start=(kc == 0), stop=(kc == KC - 1))
                    # softmax normalize (Z replicated 32x at partitions 64:96)
                    zsa = wkC.tile([HD, TQ], F32, tag="zsa")
                    nc.scalar.copy(zsa[0:32, :], p2a[HD:VW, :])
                    nc.scalar.copy(zsa[32:HD, :], zsa[0:32, :])
                    rza = wkC.tile([HD, TQ], F32, tag="rza")
                    nc.vector.reciprocal(rza, zsa)
                    nc.vector.tensor_mul(osb[0:HD, j, :], p2a[0:HD, :], rza)
                    zsb = wkC.tile([HD, TQ], F32, tag="zsb")
                    nc.scalar.copy(zsb[0:32, :], p2b[HD:VW, :])
                    nc.scalar.copy(zsb[32:HD, :], zsb[0:32, :])
                    rzb = wkC.tile([HD, TQ], F32, tag="rzb")
                    nc.vector.reciprocal(rzb, zsb)
                    onb = wkC.tile([HD, TQ], F32, tag="onb")
                    nc.vector.tensor_mul(onb, p2b[0:HD, :], rzb)
                    nc.scalar.copy(osb[HD:P, j, :], onb)

            attn_cm.__exit__(None, None, None)  # free vsb/qsb/ksb (96KB/part)

            # ================= Phase D: proj + residual ==========================
            with ExitStack() as phD:
                wpp = phD.enter_context(tc.tile_pool(name=f"wpp{rep}", bufs=3))
                psD = phD.enter_context(tc.tile_pool(name=f"psD{rep}", bufs=3, space="PSUM"))
                for fj in range(KC):
                    wt = wpp.tile([P, KC, P], BF16, tag="wpt")
                    nc.sync.dma_start(
                        out=wt, in_=d_wp[fj].rearrange("p (kc f) -> p kc f", kc=KC))
                    psp = psD.tile([P, TQ], F32, tag="ps_p")
                    for dj in range(KC):
                        nc.tensor.matmul(psp, lhsT=wt[:, dj, :], rhs=osb[:, dj, :],
                                         start=(dj == 0), stop=(dj == KC - 1))
                    # resid = (psp + b_proj) + x
                    nc.vector.scalar_tensor_tensor(
                        out=resid[:, fj, :], in0=psp, scalar=bp[:, fj:fj + 1],
                        in1=xloc[:, fj, :], op0=OP.add, op1=OP.add)

            # h2 reuses xloc's slot (t16a) -- xloc dead after phase D
            h2 = big.tile([P, KC, TQ], BF16, tag="t16a")

            # ================= Phase E: LN2 ======================================
            with ExitStack() as phE:
                wkE = phE.enter_context(tc.tile_pool(name=f"wkE{rep}", bufs=3))
                psE = phE.enter_context(tc.tile_pool(name=f"psE{rep}", bufs=1, space="PSUM"))
                m2 = wkE.tile([P, TQ], F32, tag="m2rep")
                r2 = wkE.tile([P, TQ], F32, tag="r2rep")

                def src2(kc, hf):
                    return resid[:, kc, :]

                ln_stats(src2, TQ, psE, wkE, m2, r2)
                for kc in range(KC):
                    t1 = wkE.tile([P, TQ], F32, tag="ln_t1")
                    nc.vector.tensor_sub(t1, resid[:, kc, :], m2)
                    nc.vector.tensor_mul(t1, t1, r2)
                    nc.vector.tensor_scalar(
                        out=h2[:, kc, :], in0=t1,
                        scalar1=ln2g[:, kc:kc + 1], scalar2=ln2b[:, kc:kc + 1],
                        op0=OP.mult, op1=OP.add)

            # ================= Phase F: fc1 + gelu ===============================
            gsb_cm = tc.tile_pool(name=f"gsbp{rep}", bufs=1)
            pG = gsb_cm.__enter__()
            gsb = pG.tile([P, HJ, TQ], BF16, tag="gsb")        # 64KB/part
            with ExitStack() as phF:
                wf1p = phF.enter_context(tc.tile_pool(name=f"wf1p{rep}", bufs=3))
                psF = phF.enter_context(tc.tile_pool(name=f"psF{rep}", bufs=3, space="PSUM"))
                for hj in range(HJ):
                    wt = wf1p.tile([P, KC, P], BF16, tag="wf1t")
                    nc.sync.dma_start(
                        out=wt, in_=d_wf1[hj].rearrange("p (kc f) -> p kc f", kc=KC))
                    psf = psF.tile([P, TQ], F32, tag="ps_f1")
                    for kc in range(KC):
                        nc.tensor.matmul(psf, lhsT=wt[:, kc, :], rhs=h2[:, kc, :],
                                         start=(kc == 0), stop=(kc == KC - 1))
                    nc.scalar.activation(gsb[:, hj, :], psf, FT.Gelu,
                                         bias=bf1[:, hj:hj + 1])

            # ================= Phase G: fc2 + residual + store ===================
            with ExitStack() as phG:
                wf2p = phG.enter_context(tc.tile_pool(name=f"wf2p{rep}", bufs=2))
                psG = phG.enter_context(tc.tile_pool(name=f"psG{rep}", bufs=3, space="PSUM"))
                wkG = phG.enter_context(tc.tile_pool(name=f"wkG{rep}", bufs=3))
                for fj in range(KC):
                    wt = wf2p.tile([P, HJ, P], BF16, tag="wf2t")
                    nc.sync.dma_start(
                        out=wt, in_=d_wf2[fj].rearrange("p (hj f) -> p hj f", hj=HJ))
                    psf2 = psG.tile([P, TQ], F32, tag="ps_f2")
                    for hj in range(HJ):
                        nc.tensor.matmul(psf2, lhsT=wt[:, hj, :], rhs=gsb[:, hj, :],
                                         start=(hj == 0), stop=(hj == HJ - 1))
                    ot = wkG.tile([P, TQ], F32, tag="outt")
                    nc.vector.scalar_tensor_tensor(
                        out=ot, in0=psf2, scalar=bf2[:, fj:fj + 1],
                        in1=resid[:, fj, :], op0=OP.add, op1=OP.add)
                    nc.sync.dma_start(out=d_out[fj], in_=ot)
            gsb_cm.__exit__(None, None, None)
            big.release()

        for rep in range(reps):
            emit(rep)

    return nc


# ----------------------------------------------------------------------------
# Host-side input prep
# ----------------------------------------------------------------------------

def _qk_perm():
    """Column permutation for w_q / w_k: feature-tile j holds heads 2j, 2j+1 as
    [re(2j) | im(2j) | re(2j+1) | im(2j+1)] blocks of 32."""
    j = np.arange(KC)[:, None, None]
    quad = np.arange(4)[None, :, None]
    i = np.arange(32)[None, None, :]
    src = (2 * j + quad // 2) * HD + 2 * i + (quad % 2)
    return src.reshape(-1)


def _tile_w(w, n_out_tiles):
    """[Cin, Cout] -> [n_out_tiles, 128, (Cin/128)*128]: per out-tile, the
    stationary blocks for every contraction chunk, contiguous."""
    cin = w.shape[0]
    kci = cin // P
    return np.ascontiguousarray(
        w.reshape(kci, P, n_out_tiles, P).transpose(2, 1, 0, 3).reshape(
            n_out_tiles, P, kci * P))


def _col(v):
    """[n*128] per-feature vector -> [128, n] per-partition columns."""
    return np.ascontiguousarray(v.reshape(-1, P).T)


_CACHE = {}


def _prep_shared(w_qkv, w_proj, b_proj, w_fc1, b_fc1, w_fc2, b_fc2,
                 ln1_g, ln1_b, ln2_g, ln2_b):
    perm = _qk_perm()
    wq = np.ascontiguousarray(w_qkv[:, 0 * C:1 * C][:, perm])
    wk = np.ascontiguousarray(w_qkv[:, 1 * C:2 * C][:, perm])
    wv = w_qkv[:, 2 * C:3 * C]
    shared = {}
    shared["onesT"] = np.ones((P, H * 32), np.float32)
    shared["wq"] = _tile_w(wq, KC).astype(ml_dtypes.bfloat16)
    shared["wk"] = _tile_w(wk, KC).astype(ml_dtypes.bfloat16)
    # wv is a moving operand -> [p, kc, Cout]
    shared["wv"] = np.ascontiguousarray(wv.reshape(KC, P, C).transpose(1, 0, 2)).astype(ml_dtypes.bfloat16)
    shared["wp"] = _tile_w(w_proj, KC).astype(ml_dtypes.bfloat16)
    shared["wf1"] = _tile_w(w_fc1, HJ).astype(ml_dtypes.bfloat16)
    shared["wf2"] = _tile_w(w_fc2, KC).astype(ml_dtypes.bfloat16)
    shared["ln1g"] = _col(ln1_g)
    shared["ln1b"] = _col(ln1_b)
    shared["ln2g"] = _col(ln2_g)
    shared["ln2b"] = _col(ln2_b)
    shared["bp"] = _col(b_proj)
    shared["bf1"] = _col(b_fc1)
    shared["bf2"] = _col(b_fc2)
    return shared


def make_in_maps(x, freqs_cos, freqs_sin, shared):
    # sign pattern: +sin on re-rows (0:32, 64:96), -sin on im-rows
    sgn = np.repeat(np.array([1.0, -1.0, 1.0, -1.0], np.float32), 32)[:, None]
    in_maps = []
    for c in range(NCORES):
        b, h = divmod(c, 2)
        order = np.r_[h * TQ:(h + 1) * TQ, (1 - h) * TQ:(2 - h) * TQ]
        xT = np.ascontiguousarray(x[b].T[:, order])
        cosR = np.ascontiguousarray(np.tile(freqs_cos[b].T, (4, 1))[:, order])
        sinPM = np.ascontiguousarray(
            (np.tile(freqs_sin[b].T, (4, 1)) * sgn)[:, order])
        m = {"xT": xT, "cosR": cosR, "sinPM": sinPM}
        m.update(shared)
        in_maps.append(m)
    return in_maps


def prep_all(x, freqs_cos, freqs_sin, ln1_g, ln1_b, w_qkv, w_proj, b_proj,
             ln2_g, ln2_b, w_fc1, b_fc1, w_fc2, b_fc2):
    shared = _prep_shared(
        np.asarray(w_qkv, np.float32), np.asarray(w_proj, np.float32),
        np.asarray(b_proj, np.float32), np.asarray(w_fc1, np.float32),
        np.asarray(b_fc1, np.float32), np.asarray(w_fc2, np.float32),
        np.asarray(b_fc2, np.float32), np.asarray(ln1_g, np.float32),
        np.asarray(ln1_b, np.float32), np.asarray(ln2_g, np.float32),
        np.asarray(ln2_b, np.float32))
    return make_in_maps(np.asarray(x, np.float32),
                        np.asarray(freqs_cos, np.float32),
                        np.asarray(freqs_sin, np.float32), shared)


def gather_out(results):
    out = np.empty((B, N, C), np.float32)
    for c in range(NCORES):
        b, h = divmod(c, 2)
        outT = np.asarray(results[c]["outT"]).reshape(C, TQ)
        out[b, h * TQ:(h + 1) * TQ, :] = outT.T
    return out


def kernel(x, freqs_cos, freqs_sin, ln1_g, ln1_b, w_qkv, w_proj, b_proj,
           ln2_g, ln2_b, w_fc1, b_fc1, w_fc2, b_fc2):
    _install_multiwait_hook()
    if "nc" not in _CACHE:
        _CACHE["nc"] = build_nc()
    nc = _CACHE["nc"]
    in_maps = prep_all(x, freqs_cos, freqs_sin, ln1_g, ln1_b, w_qkv, w_proj,
                       b_proj, ln2_g, ln2_b, w_fc1, b_fc1, w_fc2, b_fc2)
    res = run_bass_kernel_spmd(nc, in_maps, core_ids=list(range(NCORES)))
    return gather_out(res.results)

